# revision 34
# baseline (speedup 1.0000x reference)
"""Trainium2 Bass kernel for nn_CausalFieldAttention.

Shapes (hardcoded): B=4, N=4096, D=1024, H=16, hd=64, G=512, sigma=3.

Reference computation (the q-projection is computed but unused -> skipped):
    k  = x @ k_w.T + k_b                      (B,N,D) -> heads (B,H,N,hd)
    v  = x @ v_w.T + v_b
    wv = v * ||k||_head                       per-token, per-head scale
    field = segment_sum(wv, field_idx, G)     scatter tokens -> G bins
    conv  = circular_conv(field, causal_ker)  (reference: via rfft/irfft)
    y  = conv[field_idx]                      gather bins -> tokens
    out = y @ out_w.T + out_b

Device strategy: 8 cores = 4 batches x 2 head-groups (8 heads / 512 channels
each), everything in f32r (full-rate fp32 matmul mode):
  - k/v projections: (tok x ch) psum tiles, contraction over D.
  - ||k||: one ACT Square per token tile + DVE grouped reduce + ACT sqrt;
    wv = v * ||k|| as one DVE multiply with a stride-0 broadcast AP.
  - scatter: block-sparse 0/1 matrix S; tokens are sorted by bin, so each
    128-token tile hits ~17 consecutive bins => ~1 matmul per tile.
  - circular conv: exact circulant matmul, produced transposed:
    convT = field.T @ C.T (the FFT in the reference is just this, exactly).
  - KEY reassociation: out = gather(conv) @ out_w = gather(conv @ out_w).
    A = conv @ ow is computed once at bin granularity (512 rows instead of
    4096), then the gather IS the final matmul: out(t,e) = S.T @ A.
  - out-projection partial per core over its 512 channels; host sums the
    two head-group partials per batch and adds out_b.
"""

import os
import sys
from contextlib import ExitStack

import numpy as np

for _p in ("/opt/trn_rl_repo", "/root/.axon_site/_ro/trn_rl_repo"):
    if os.path.isdir(_p) and _p not in sys.path:
        sys.path.append(_p)

import concourse.bacc as bacc
import concourse.mybir as mybir
import concourse.tile as tile
from concourse.bass_utils import run_bass_kernel_spmd

B, N, D = 4, 4096, 1024
H, HD, G = 16, 64, 512
SIGMA = 3.0
P = 128
KT = D // P          # 8 contraction tiles over D
TT = N // P          # 32 token tiles
GT = G // P          # 4 bin tiles
CLOC = 512           # channels per core (8 heads)
HLOC = CLOC // HD    # 8 heads per core
ECH = D // 512       # 2 chunks of out-channels for 512-wide psum
NCORES = 8

F32 = mybir.dt.float32
F32R = mybir.dt.float32r

# set by test harness to capture a profile; kernel() stores results here
TRACE = False
LAST_RESULT = None


def _field_idx():
    # exactly mirrors the reference (fp32 div then mul, trunc, clip)
    pos = np.arange(N, dtype=np.float32) / np.float32(N - 1) * np.float32(G - 1)
    return np.clip(pos.astype(np.int32), 0, G - 1)


def _causal_kernel():
    i = np.arange(G)
    dist = np.abs(i - G // 2)
    ker = np.where(i >= G // 2, 0.0, np.exp(-dist / SIGMA)).astype(np.float32)
    ker = ker / (ker.sum() + 1e-8)
    return ker


def _plans():
    idx = _field_idx()
    ker = _causal_kernel()
    gg = (np.arange(G)[None, :] - np.arange(G)[:, None]) % G  # CT[g, g2] = ker[(g2-g)%G]
    CTm = ker[gg].astype(np.float32)

    Smat = np.zeros((N, G), np.float32)
    Smat[np.arange(N), idx] = 1.0
    STm = np.ascontiguousarray(Smat.T)

    tt_gts = [sorted(set((idx[t * P:(t + 1) * P] // P).tolist())) for t in range(TT)]
    contribs = {gt: [t for t in range(TT) if gt in tt_gts[t]] for gt in range(GT)}
    scatter_plan = [
        [(gt, t == contribs[gt][0], t == contribs[gt][-1]) for gt in tt_gts[t]]
        for t in range(TT)
    ]
    conv_blocks = [
        [gt for gt in range(GT)
         if np.abs(CTm[gt * P:(gt + 1) * P, gp * P:(gp + 1) * P]).max() > 1e-12]
        for gp in range(GT)
    ]
    return idx, CTm, Smat, STm, scatter_plan, tt_gts, conv_blocks


def _build_program(with_kb, with_vb, scatter_plan, tt_gts, conv_blocks):
    nc = bacc.Bacc("TRN2", target_bir_lowering=False, debug=False,
                   num_devices=NCORES)
    xT = nc.dram_tensor("xT", [D, N], F32R, kind="ExternalInput").ap()
    kwT = nc.dram_tensor("kwT", [D, CLOC], F32R, kind="ExternalInput").ap()
    vwT = nc.dram_tensor("vwT", [D, CLOC], F32R, kind="ExternalInput").ap()
    owT = nc.dram_tensor("owT", [CLOC, D], F32R, kind="ExternalInput").ap()
    Sm = nc.dram_tensor("Smat", [N, G], F32R, kind="ExternalInput").ap()
    STmat = nc.dram_tensor("STm", [G, N], F32R, kind="ExternalInput").ap()
    CTmat = nc.dram_tensor("CTm", [G, G], F32R, kind="ExternalInput").ap()
    kb = nc.dram_tensor("kb", [1, CLOC], F32R, kind="ExternalInput").ap() if with_kb else None
    vb = nc.dram_tensor("vb", [1, CLOC], F32R, kind="ExternalInput").ap() if with_vb else None
    ones_d = (nc.dram_tensor("ones", [1, P], F32R, kind="ExternalInput").ap()
              if (with_kb or with_vb) else None)
    out_d = nc.dram_tensor("out", [N, D], F32, kind="ExternalOutput").ap()

    xT_r = xT.rearrange("(kt p) n -> p kt n", p=P)
    kwT_r = kwT.rearrange("(kt p) c -> p kt c", p=P)
    vwT_r = vwT.rearrange("(kt p) c -> p kt c", p=P)

    with tile.TileContext(nc) as tc, ExitStack() as es:
        cpool = es.enter_context(tc.tile_pool(name="const", bufs=1))

        # resident tensors; k/v weights split per-kt so the first projection
        # matmuls only wait on their own 256KB slice (subtile deps).
        # Queue order matters: the HWDGE queues drain in issue order, so the
        # first token tile's x block goes out first, then weights round-robin
        # over the three DMA-capable queues; ow/ct are deferred to mid-loop.
        kw_sb = cpool.tile([P, KT, CLOC], F32R)
        vw_sb = cpool.tile([P, KT, CLOC], F32R)
        ow_sb = cpool.tile([P, GT, D], F32R)
        ct_sb = cpool.tile([P, GT, G], F32R)
        field_sb = cpool.tile([P, GT, G], F32R)
        convT_sb = cpool.tile([P, GT, G], F32R)
        A_sb = cpool.tile([P, GT, D], F32R)
        if with_kb or with_vb:
            ones_sb = cpool.tile([1, P], F32R)
            nc.sync.dma_start(ones_sb[:], ones_d[:])
        if with_kb:
            kb_sb = cpool.tile([1, CLOC], F32R)
            nc.sync.dma_start(kb_sb[:], kb[:])
        if with_vb:
            vb_sb = cpool.tile([1, CLOC], F32R)
            nc.sync.dma_start(vb_sb[:], vb[:])

        stpool = es.enter_context(tc.tile_pool(name="st_in", bufs=1))
        opool = es.enter_context(tc.tile_pool(name="osb", bufs=4))
        st_tiles = {tt: {} for tt in range(TT)}
        st_jobs = []
        for tt in range(TT):
            for gt in tt_gts[tt]:
                st = stpool.tile([P, P], F32R, tag=f"st_{tt}_{gt}",
                                 name=f"st_{tt}_{gt}")
                st_tiles[tt][gt] = st
                st_jobs.append((tt, gt, st))

        # ---- phase 1: projections, ||k||, wv, scatter ----
        ph1 = ExitStack()
        xpool = ph1.enter_context(tc.tile_pool(name="xin", bufs=3))
        spool = ph1.enter_context(tc.tile_pool(name="sblk", bufs=4))
        wvpool = ph1.enter_context(tc.tile_pool(name="wv", bufs=4))
        smpool = ph1.enter_context(tc.tile_pool(name="small", bufs=3))
        ps_k = ph1.enter_context(tc.tile_pool(name="ps_k", bufs=2, space="PSUM"))
        ps_v = ph1.enter_context(tc.tile_pool(name="ps_v", bufs=2, space="PSUM"))
        ps_f = ph1.enter_context(tc.tile_pool(name="ps_f", bufs=2, space="PSUM"))
        ps_mid = ph1.enter_context(tc.tile_pool(name="ps_mid", bufs=1, space="PSUM"))

        field_ps = {}

        # ---- mid-stage jobs: convT column-tiles, A slices, and token-tile
        # output writes, emitted inside phase 1 as their field deps complete.
        # conv_blocks[gp] lists the only bin-tiles feeding convT[:, gp] (the
        # causal kernel's support), so gp=2 is ready after field gt<=1, gp=3
        # after gt<=2; gp=0,1 wrap circularly and must wait for the end.
        def job_convT(gp, pool, tag):
            def run():
                mt = pool.tile([P, D], F32, tag=tag, name=f"cvt{gp}")
                blocks = conv_blocks[gp]
                for ct in range(GT):
                    for gi, gt in enumerate(blocks):
                        nc.tensor.matmul(
                            mt[:, ct * P:(ct + 1) * P],
                            field_sb[:, gt, ct * P:(ct + 1) * P],
                            ct_sb[:, gt, gp * P:(gp + 1) * P],
                            start=(gi == 0), stop=(gi == len(blocks) - 1))
                eng = nc.vector if gp % 2 == 0 else nc.scalar
                if gp % 2 == 0:
                    nc.vector.tensor_copy(
                        convT_sb[:, :, gp * P:(gp + 1) * P],
                        mt[:, 0:G].rearrange("p (ct f) -> p ct f", f=P))
                else:
                    nc.scalar.copy(
                        convT_sb[:, :, gp * P:(gp + 1) * P],
                        mt[:, 0:G].rearrange("p (ct f) -> p ct f", f=P))
            return run

        def job_A(gp, pool, tag):
            def run():
                mt = pool.tile([P, D], F32, tag=tag, name=f"amt{gp}")
                for ec in range(ECH):
                    esl = slice(ec * 512, (ec + 1) * 512)
                    for ct in range(GT):
                        nc.tensor.matmul(mt[:, esl],
                                         convT_sb[:, ct, gp * P:(gp + 1) * P],
                                         ow_sb[:, ct, esl],
                                         start=(ct == 0), stop=(ct == GT - 1))
                if gp % 2 == 0:
                    nc.vector.tensor_copy(A_sb[:, gp, :], mt[:])
                else:
                    nc.scalar.copy(A_sb[:, gp, :], mt[:])
            return run

        def job_out(tt, pool, tag):
            def run():
                tsl = slice(tt * P, (tt + 1) * P)
                gts = tt_gts[tt]
                mt = pool.tile([P, D], F32, tag=tag, name=f"omt{tt}")
                for ec in range(ECH):
                    esl = slice(ec * 512, (ec + 1) * 512)
                    for i, gt in enumerate(gts):
                        nc.tensor.matmul(mt[:, esl], st_tiles[tt][gt][:],
                                         A_sb[:, gt, esl],
                                         start=(i == 0), stop=(i == len(gts) - 1))
                osb = opool.tile([P, D], F32, tag="osb")
                if tt % 3 == 0:
                    nc.scalar.copy(osb[:], mt[:])
                else:
                    nc.vector.tensor_copy(osb[:], mt[:])
                nc.sync.dma_start(out_d[tsl, :], osb[:])
            return run

        # enqueue points: field copy for gt lands during iteration
        # (last_contrib(gt) + 1) via the pending-scatter delay
        last_tt = {gt: max(t for t in range(TT) if gt in tt_gts[t])
                   for gt in range(GT)}
        enqueue_at = {}
        ready2 = last_tt[1] + 2      # field gt0,gt1 copied
        ready3 = last_tt[2] + 2
        enqueue_at.setdefault(ready2, []).append(("cvt", 2))
        enqueue_at.setdefault(ready2 + 1, []).append(("A", 2))
        enqueue_at.setdefault(ready3, []).append(("cvt", 3))
        enqueue_at.setdefault(ready3 + 1, []).append(("A", 3))
        for t in range(TT):
            if set(tt_gts[t]) <= {2}:
                enqueue_at.setdefault(ready2 + 2, []).append(("out", t))
            elif set(tt_gts[t]) <= {2, 3}:
                enqueue_at.setdefault(ready3 + 2, []).append(("out", t))
        mid_queue = []

        def emit_scatter(tt, wv):
            tsl = slice(tt * P, (tt + 1) * P)
            for gt, first, last in scatter_plan[tt]:
                if first:
                    field_ps[gt] = ps_f.tile([P, CLOC], F32, tag="fld",
                                             name=f"fld{gt}")
                sblk = spool.tile([P, P], F32R, tag="sblk")
                nc.gpsimd.dma_start(sblk[:], Sm[tsl, gt * P:(gt + 1) * P])
                nc.tensor.matmul(field_ps[gt][:], sblk[:], wv[:],
                                 start=first, stop=last)
                if last:
                    if gt % 2 == 0:
                        nc.vector.tensor_copy(field_sb[:, gt, :], field_ps[gt][:])
                    else:
                        nc.scalar.copy(field_sb[:, gt, :], field_ps[gt][:])

        pending = None
        xb_pre = {tt: xpool.tile([P, KT, P], F32R, tag="xblk", bufs=5,
                                 name=f"xb{tt}") for tt in range(4)}
        # startup: deadline-ordered issue across the three DMA queues so
        # operands land in PE consumption order (kps kt=0..7, vps kt=0..7,
        # then the next token tiles)
        def xb0(kt):
            return (xb_pre[0][:, kt, :], xT_r[:, kt, 0:P])
        def kw(kt):
            return (kw_sb[:, kt, :], kwT_r[:, kt, :])
        def vw(kt):
            return (vw_sb[:, kt, :], vwT_r[:, kt, :])
        def xbf(tt):
            return (xb_pre[tt][:], xT_r[:, :, tt * P:(tt + 1) * P])
        plan = {
            nc.sync:   [xb0(0), kw(0), kw(3), vw(2), kw(6), vw(5), vw(7), xbf(3)],
            nc.scalar: [xb0(1), kw(1), kw(4), vw(0), kw(7), vw(3), vw(6)],
            nc.gpsimd: [xb0(2), kw(2), xb0(3), xb0(4), kw(5), xb0(5), xb0(6),
                        xb0(7), vw(1), xbf(1), vw(4), xbf(2)],
        }
        for eng, items in plan.items():
            for dst, srcap in items:
                eng.dma_start(dst, srcap)
        for tt in range(TT):
            tsl = slice(tt * P, (tt + 1) * P)
            if tt in xb_pre:
                xb = xb_pre[tt]
            else:
                xb = xpool.tile([P, KT, P], F32R, tag="xblk", bufs=5, name="xb")
                nc.sync.dma_start(xb[:], xT_r[:, :, tsl])
            if tt == 8:
                # phase-2/3 constants, needed much later
                nc.gpsimd.dma_start(ow_sb[:], owT.rearrange("(ct p) e -> p ct e", p=P))
                nc.gpsimd.dma_start(ct_sb[:], CTmat.rearrange("(gt p) g2 -> p gt g2", p=P))

            kps = ps_k.tile([P, CLOC], F32, tag="kps")
            vps = ps_v.tile([P, CLOC], F32, tag="vps")
            for kt in range(KT):
                nc.tensor.matmul(kps[:], xb[:, kt, :], kw_sb[:, kt, :],
                                 start=(kt == 0), stop=(kt == KT - 1 and not with_kb))
            for kt in range(KT):
                nc.tensor.matmul(vps[:], xb[:, kt, :], vw_sb[:, kt, :],
                                 start=(kt == 0), stop=(kt == KT - 1 and not with_vb))
            if with_kb:
                nc.tensor.matmul(kps[:], ones_sb[:], kb_sb[:], start=False, stop=True)
            if with_vb:
                nc.tensor.matmul(vps[:], ones_sb[:], vb_sb[:], start=False, stop=True)

            # scatter for the previous tile (keeps PE dense: its wv is ready)
            if pending is not None:
                emit_scatter(*pending)
            if tt >= 10:
                i0 = (tt - 10) * 3
                for stt, sgt, st in st_jobs[i0:i0 + 3]:
                    nc.gpsimd.dma_start(
                        st[:], STmat[sgt * P:(sgt + 1) * P, stt * P:(stt + 1) * P])
            for kind, arg in enqueue_at.get(tt, []):
                mid_queue.append((kind, arg))
            for _ in range(2):
                if mid_queue:
                    kind, arg = mid_queue.pop(0)
                    mk = {"cvt": job_convT, "A": job_A, "out": job_out}[kind]
                    mk(arg, ps_mid, "mid")()

            # ||k|| per head
            ksq = smpool.tile([P, CLOC], F32, tag="ksq")
            nc.scalar.activation(ksq[:], kps[:], mybir.ActivationFunctionType.Square)
            km2 = smpool.tile([P, HLOC], F32, tag="km2")
            nc.vector.reduce_sum(km2[:], ksq[:].rearrange("p (h d) -> p h d", d=HD),
                                 axis=mybir.AxisListType.X)
            km = smpool.tile([P, HLOC], F32, tag="km")
            nc.scalar.sqrt(km[:], km2[:])

            # wv = v * ||k||, one DVE op via stride-0 broadcast of km
            wv = wvpool.tile([P, CLOC], F32R, tag="wv")
            nc.vector.tensor_tensor(
                wv[:].rearrange("p (h d) -> p h d", d=HD),
                vps[:].rearrange("p (h d) -> p h d", d=HD),
                km[:].unsqueeze(2).broadcast_to((P, HLOC, HD)),
                mybir.AluOpType.mult)
            pending = (tt, wv)

        emit_scatter(*pending)
        ph1.close()

        # flush any queued mid jobs (still inside ph1 pools)
        while mid_queue:
            kind, arg = mid_queue.pop(0)
            mk = {"cvt": job_convT, "A": job_A, "out": job_out}[kind]
            mk(arg, ps_mid, "mid")()
        ph1.close()

        # ---- tail: circular-wrap convT tiles 0,1 -> A -> remaining tokens ----
        ph2 = ExitStack()
        ps_t = ph2.enter_context(tc.tile_pool(name="ps_t", bufs=3, space="PSUM"))
        done = {t for jobs in enqueue_at.values() for k, t in jobs if k == "out"}
        for gp in (0, 1):
            job_convT(gp, ps_t, "tmid")()
            job_A(gp, ps_t, "tmid")()
        for t in range(TT):
            if t not in done:
                job_out(t, ps_t, "tmid")()
        ph2.close()

    nc.compile()
    return nc


_PROGRAM_CACHE = {}


def _get_program(with_kb, with_vb):
    key = (with_kb, with_vb)
    if key not in _PROGRAM_CACHE:
        _, _, _, _, sp, tg, cb = _plans()
        _PROGRAM_CACHE[key] = _build_program(with_kb, with_vb, sp, tg, cb)
    return _PROGRAM_CACHE[key]


def kernel(x, q_w, q_b, k_w, k_b, v_w, v_b, out_w, out_b):
    global LAST_RESULT
    x = np.asarray(x, dtype=np.float32)
    k_w = np.asarray(k_w, dtype=np.float32)
    k_b = np.asarray(k_b, dtype=np.float32)
    v_w = np.asarray(v_w, dtype=np.float32)
    v_b = np.asarray(v_b, dtype=np.float32)
    out_w = np.asarray(out_w, dtype=np.float32)
    out_b = np.asarray(out_b, dtype=np.float32)

    with_kb = bool(np.any(k_b))
    with_vb = bool(np.any(v_b))
    nc = _get_program(with_kb, with_vb)
    _, CTm, Smat, STm, _, _, _ = _plans()

    in_maps = []
    for c in range(NCORES):
        b, hg = c // 2, c % 2
        chs = slice(hg * CLOC, (hg + 1) * CLOC)
        m = {
            "xT": np.ascontiguousarray(x[b].T),
            "kwT": np.ascontiguousarray(k_w[chs, :].T),
            "vwT": np.ascontiguousarray(v_w[chs, :].T),
            "owT": np.ascontiguousarray(out_w[:, chs].T),
            "Smat": Smat,
            "STm": STm,
            "CTm": CTm,
        }
        if with_kb:
            m["kb"] = np.ascontiguousarray(k_b[chs][None, :])
        if with_vb:
            m["vb"] = np.ascontiguousarray(v_b[chs][None, :])
        if with_kb or with_vb:
            m["ones"] = np.ones((1, P), dtype=np.float32)
        in_maps.append(m)

    res = run_bass_kernel_spmd(nc, in_maps, core_ids=list(range(NCORES)),
                               trace=TRACE)
    LAST_RESULT = res

    out = np.empty((B, N, D), dtype=np.float32)
    for b in range(B):
        out[b] = res.results[2 * b]["out"] + res.results[2 * b + 1]["out"]
        out[b] += out_b[None, :]
    return out


# revision 35
# speedup vs baseline: 1.1422x; 1.1422x over previous
"""Trainium2 Bass kernel for nn_CausalFieldAttention.

Shapes (hardcoded): B=4, N=4096, D=1024, H=16, hd=64, G=512, sigma=3.

Reference computation (the q-projection is computed but unused -> skipped):
    k  = x @ k_w.T + k_b                      (B,N,D) -> heads (B,H,N,hd)
    v  = x @ v_w.T + v_b
    wv = v * ||k||_head                       per-token, per-head scale
    field = segment_sum(wv, field_idx, G)     scatter tokens -> G bins
    conv  = circular_conv(field, causal_ker)  (reference: via rfft/irfft)
    y  = conv[field_idx]                      gather bins -> tokens
    out = y @ out_w.T + out_b

Device strategy: 8 cores = 4 batches x 2 head-groups (8 heads / 512 channels
each), everything in f32r (full-rate fp32 matmul mode):
  - k/v projections: (tok x ch) psum tiles, contraction over D.
  - ||k||: one ACT Square per token tile + DVE grouped reduce + ACT sqrt;
    wv = v * ||k|| as one DVE multiply with a stride-0 broadcast AP.
  - scatter: block-sparse 0/1 matrix S; tokens are sorted by bin, so each
    128-token tile hits ~17 consecutive bins => ~1 matmul per tile.
  - circular conv: exact circulant matmul, produced transposed:
    convT = field.T @ C.T (the FFT in the reference is just this, exactly).
  - KEY reassociation: out = gather(conv) @ out_w = gather(conv @ out_w).
    A = conv @ ow is computed once at bin granularity (512 rows instead of
    4096), then the gather IS the final matmul: out(t,e) = S.T @ A.
  - out-projection partial per core over its 512 channels; host sums the
    two head-group partials per batch and adds out_b.
"""

import os
import sys
from contextlib import ExitStack

import numpy as np

for _p in ("/opt/trn_rl_repo", "/root/.axon_site/_ro/trn_rl_repo"):
    if os.path.isdir(_p) and _p not in sys.path:
        sys.path.append(_p)

import concourse.bacc as bacc
import concourse.mybir as mybir
import concourse.tile as tile
from concourse.bass_utils import run_bass_kernel_spmd

B, N, D = 4, 4096, 1024
H, HD, G = 16, 64, 512
SIGMA = 3.0
P = 128
KT = D // P          # 8 contraction tiles over D
TT = N // P          # 32 token tiles
GT = G // P          # 4 bin tiles
CLOC = 512           # channels per core (8 heads)
HLOC = CLOC // HD    # 8 heads per core
ECH = D // 512       # 2 chunks of out-channels for 512-wide psum
NCORES = 8

F32 = mybir.dt.float32
F32R = mybir.dt.float32r

# set by test harness to capture a profile; kernel() stores results here
TRACE = False
LAST_RESULT = None


def _field_idx():
    # exactly mirrors the reference (fp32 div then mul, trunc, clip)
    pos = np.arange(N, dtype=np.float32) / np.float32(N - 1) * np.float32(G - 1)
    return np.clip(pos.astype(np.int32), 0, G - 1)


def _causal_kernel():
    i = np.arange(G)
    dist = np.abs(i - G // 2)
    ker = np.where(i >= G // 2, 0.0, np.exp(-dist / SIGMA)).astype(np.float32)
    ker = ker / (ker.sum() + 1e-8)
    return ker


def _plans():
    idx = _field_idx()
    ker = _causal_kernel()
    gg = (np.arange(G)[None, :] - np.arange(G)[:, None]) % G  # CT[g, g2] = ker[(g2-g)%G]
    CTm = ker[gg].astype(np.float32)

    Smat = np.zeros((N, G), np.float32)
    Smat[np.arange(N), idx] = 1.0
    STm = np.ascontiguousarray(Smat.T)

    tt_gts = [sorted(set((idx[t * P:(t + 1) * P] // P).tolist())) for t in range(TT)]
    contribs = {gt: [t for t in range(TT) if gt in tt_gts[t]] for gt in range(GT)}
    scatter_plan = [
        [(gt, t == contribs[gt][0], t == contribs[gt][-1]) for gt in tt_gts[t]]
        for t in range(TT)
    ]
    conv_blocks = [
        [gt for gt in range(GT)
         if np.abs(CTm[gt * P:(gt + 1) * P, gp * P:(gp + 1) * P]).max() > 1e-12]
        for gp in range(GT)
    ]
    return idx, CTm, Smat, STm, scatter_plan, tt_gts, conv_blocks


def _build_program(with_kb, with_vb, scatter_plan, tt_gts, conv_blocks):
    nc = bacc.Bacc("TRN2", target_bir_lowering=False, debug=False,
                   num_devices=NCORES)
    xT = nc.dram_tensor("xT", [D, N], F32R, kind="ExternalInput").ap()
    kwT = nc.dram_tensor("kwT", [D, CLOC], F32R, kind="ExternalInput").ap()
    vwT = nc.dram_tensor("vwT", [D, CLOC], F32R, kind="ExternalInput").ap()
    owT = nc.dram_tensor("owT", [CLOC, D], F32R, kind="ExternalInput").ap()
    Sm = nc.dram_tensor("Smat", [N, G], F32R, kind="ExternalInput").ap()
    STmat = nc.dram_tensor("STm", [G, N], F32R, kind="ExternalInput").ap()
    CTmat = nc.dram_tensor("CTm", [G, G], F32R, kind="ExternalInput").ap()
    kb = nc.dram_tensor("kb", [1, CLOC], F32R, kind="ExternalInput").ap() if with_kb else None
    vb = nc.dram_tensor("vb", [1, CLOC], F32R, kind="ExternalInput").ap() if with_vb else None
    ones_d = (nc.dram_tensor("ones", [1, P], F32R, kind="ExternalInput").ap()
              if (with_kb or with_vb) else None)
    out_d = nc.dram_tensor("out", [N, D], F32, kind="ExternalOutput").ap()

    xT_r = xT.rearrange("(kt p) n -> p kt n", p=P)
    kwT_r = kwT.rearrange("(kt p) c -> p kt c", p=P)
    vwT_r = vwT.rearrange("(kt p) c -> p kt c", p=P)

    with tile.TileContext(nc) as tc, ExitStack() as es:
        cpool = es.enter_context(tc.tile_pool(name="const", bufs=1))

        # resident tensors; k/v weights split per-kt so the first projection
        # matmuls only wait on their own 256KB slice (subtile deps).
        # Queue order matters: the HWDGE queues drain in issue order, so the
        # first token tile's x block goes out first, then weights round-robin
        # over the three DMA-capable queues; ow/ct are deferred to mid-loop.
        kw_sb = cpool.tile([P, KT, CLOC], F32R)
        vw_sb = cpool.tile([P, KT, CLOC], F32R)
        ow_sb = cpool.tile([P, GT, D], F32R)
        ct_sb = cpool.tile([P, GT, G], F32R)
        field_sb = cpool.tile([P, GT, G], F32R)
        convT_sb = cpool.tile([P, GT, G], F32R)
        A_sb = cpool.tile([P, GT, D], F32R)
        if with_kb or with_vb:
            ones_sb = cpool.tile([1, P], F32R)
            nc.sync.dma_start(ones_sb[:], ones_d[:])
        if with_kb:
            kb_sb = cpool.tile([1, CLOC], F32R)
            nc.sync.dma_start(kb_sb[:], kb[:])
        if with_vb:
            vb_sb = cpool.tile([1, CLOC], F32R)
            nc.sync.dma_start(vb_sb[:], vb[:])

        stpool = es.enter_context(tc.tile_pool(name="st_in", bufs=1))
        opool = es.enter_context(tc.tile_pool(name="osb", bufs=3))
        st_tiles = {tt: {} for tt in range(TT)}
        st_jobs = []
        for tt in range(TT):
            for gt in tt_gts[tt]:
                st = stpool.tile([P, P], F32R, tag=f"st_{tt}_{gt}",
                                 name=f"st_{tt}_{gt}")
                st_tiles[tt][gt] = st
                st_jobs.append((tt, gt, st))

        # ---- phase 1: projections, ||k||, wv, scatter ----
        ph1 = ExitStack()
        xpool = ph1.enter_context(tc.tile_pool(name="xin", bufs=3))
        spool = ph1.enter_context(tc.tile_pool(name="sblk", bufs=4))
        wvpool = ph1.enter_context(tc.tile_pool(name="wv", bufs=4))
        smpool = ph1.enter_context(tc.tile_pool(name="small", bufs=3))
        ps_k = ph1.enter_context(tc.tile_pool(name="ps_k", bufs=2, space="PSUM"))
        ps_v = ph1.enter_context(tc.tile_pool(name="ps_v", bufs=2, space="PSUM"))
        ps_f = ph1.enter_context(tc.tile_pool(name="ps_f", bufs=2, space="PSUM"))
        ps_mid = ph1.enter_context(tc.tile_pool(name="ps_mid", bufs=1, space="PSUM"))

        field_ps = {}

        # ---- mid-stage jobs: convT column-tiles, A slices, and token-tile
        # output writes, emitted inside phase 1 as their field deps complete.
        # conv_blocks[gp] lists the only bin-tiles feeding convT[:, gp] (the
        # causal kernel's support), so gp=2 is ready after field gt<=1, gp=3
        # after gt<=2; gp=0,1 wrap circularly and must wait for the end.
        def job_convT(gp, pool, tag):
            def run():
                mt = pool.tile([P, D], F32, tag=tag, name=f"cvt{gp}")
                blocks = conv_blocks[gp]
                for ct in range(GT):
                    for gi, gt in enumerate(blocks):
                        nc.tensor.matmul(
                            mt[:, ct * P:(ct + 1) * P],
                            field_sb[:, gt, ct * P:(ct + 1) * P],
                            ct_sb[:, gt, gp * P:(gp + 1) * P],
                            start=(gi == 0), stop=(gi == len(blocks) - 1))
                eng = nc.vector if gp % 2 == 0 else nc.scalar
                if gp % 2 == 0:
                    nc.vector.tensor_copy(
                        convT_sb[:, :, gp * P:(gp + 1) * P],
                        mt[:, 0:G].rearrange("p (ct f) -> p ct f", f=P))
                else:
                    nc.scalar.copy(
                        convT_sb[:, :, gp * P:(gp + 1) * P],
                        mt[:, 0:G].rearrange("p (ct f) -> p ct f", f=P))
            return run

        def job_A(gp, pool, tag):
            def run():
                mt = pool.tile([P, D], F32, tag=tag, name=f"amt{gp}")
                for ec in range(ECH):
                    esl = slice(ec * 512, (ec + 1) * 512)
                    for ct in range(GT):
                        nc.tensor.matmul(mt[:, esl],
                                         convT_sb[:, ct, gp * P:(gp + 1) * P],
                                         ow_sb[:, ct, esl],
                                         start=(ct == 0), stop=(ct == GT - 1))
                if gp % 2 == 0:
                    nc.vector.tensor_copy(A_sb[:, gp, :], mt[:])
                else:
                    nc.scalar.copy(A_sb[:, gp, :], mt[:])
            return run

        def job_out(tt, pool, tag):
            def run():
                tsl = slice(tt * P, (tt + 1) * P)
                gts = tt_gts[tt]
                mt = pool.tile([P, D], F32, tag=tag, name=f"omt{tt}")
                for ec in range(ECH):
                    esl = slice(ec * 512, (ec + 1) * 512)
                    for i, gt in enumerate(gts):
                        nc.tensor.matmul(mt[:, esl], st_tiles[tt][gt][:],
                                         A_sb[:, gt, esl],
                                         start=(i == 0), stop=(i == len(gts) - 1))
                osb = opool.tile([P, D], F32, tag="osb")
                if tt % 3 == 0:
                    nc.scalar.copy(osb[:], mt[:])
                else:
                    nc.vector.tensor_copy(osb[:], mt[:])
                nc.sync.dma_start(out_d[tsl, :], osb[:])
            return run

        # enqueue points: field copy for gt lands during iteration
        # (last_contrib(gt) + 1) via the pending-scatter delay
        last_tt = {gt: max(t for t in range(TT) if gt in tt_gts[t])
                   for gt in range(GT)}
        enqueue_at = {}
        ready2 = last_tt[1] + 2      # field gt0,gt1 copied
        ready3 = last_tt[2] + 2
        enqueue_at.setdefault(ready2, []).append(("cvt", 2))
        enqueue_at.setdefault(ready2 + 1, []).append(("A", 2))
        enqueue_at.setdefault(ready3, []).append(("cvt", 3))
        enqueue_at.setdefault(ready3 + 1, []).append(("A", 3))
        for t in range(TT):
            if set(tt_gts[t]) <= {2}:
                enqueue_at.setdefault(ready2 + 2, []).append(("out", t))
            elif set(tt_gts[t]) <= {2, 3}:
                enqueue_at.setdefault(ready3 + 2, []).append(("out", t))
        mid_queue = []

        def emit_scatter(tt, wv):
            tsl = slice(tt * P, (tt + 1) * P)
            for gt, first, last in scatter_plan[tt]:
                if first:
                    field_ps[gt] = ps_f.tile([P, CLOC], F32, tag="fld",
                                             name=f"fld{gt}")
                sblk = spool.tile([P, P], F32R, tag="sblk")
                nc.gpsimd.dma_start(sblk[:], Sm[tsl, gt * P:(gt + 1) * P])
                nc.tensor.matmul(field_ps[gt][:], sblk[:], wv[:],
                                 start=first, stop=last)
                if last:
                    if gt % 2 == 0:
                        nc.vector.tensor_copy(field_sb[:, gt, :], field_ps[gt][:])
                    else:
                        nc.scalar.copy(field_sb[:, gt, :], field_ps[gt][:])

        pending = None
        xb_pre = {tt: xpool.tile([P, KT, P], F32R, tag="xblk", bufs=5,
                                 name=f"xb{tt}") for tt in range(4)}
        # startup: deadline-ordered issue across the three DMA queues so
        # operands land in PE consumption order (kps kt=0..7, vps kt=0..7,
        # then the next token tiles)
        def xb0(kt):
            return (xb_pre[0][:, kt, :], xT_r[:, kt, 0:P])
        def kw(kt):
            return (kw_sb[:, kt, :], kwT_r[:, kt, :])
        def vw(kt):
            return (vw_sb[:, kt, :], vwT_r[:, kt, :])
        def xbf(tt):
            return (xb_pre[tt][:], xT_r[:, :, tt * P:(tt + 1) * P])
        plan = {
            nc.sync:   [xb0(0), kw(0), kw(3), vw(2), kw(6), vw(5), vw(7), xbf(3)],
            nc.scalar: [xb0(1), kw(1), kw(4), vw(0), kw(7), vw(3), vw(6)],
            nc.gpsimd: [xb0(2), kw(2), xb0(3), xb0(4), kw(5), xb0(5), xb0(6),
                        xb0(7), vw(1), xbf(1), vw(4), xbf(2)],
        }
        for eng, items in plan.items():
            for dst, srcap in items:
                eng.dma_start(dst, srcap)
        for tt in range(TT):
            tsl = slice(tt * P, (tt + 1) * P)
            if tt in xb_pre:
                xb = xb_pre[tt]
            else:
                xb = xpool.tile([P, KT, P], F32R, tag="xblk", bufs=5, name="xb")
                nc.sync.dma_start(xb[:], xT_r[:, :, tsl])
            if tt == 8:
                # phase-2/3 constants, needed much later
                nc.gpsimd.dma_start(ow_sb[:], owT.rearrange("(ct p) e -> p ct e", p=P))
                nc.gpsimd.dma_start(ct_sb[:], CTmat.rearrange("(gt p) g2 -> p gt g2", p=P))

            kps = ps_k.tile([P, CLOC], F32, tag="kps")
            vps = ps_v.tile([P, CLOC], F32, tag="vps")
            for kt in range(KT):
                nc.tensor.matmul(kps[:], xb[:, kt, :], kw_sb[:, kt, :],
                                 start=(kt == 0), stop=(kt == KT - 1 and not with_kb))
            for kt in range(KT):
                nc.tensor.matmul(vps[:], xb[:, kt, :], vw_sb[:, kt, :],
                                 start=(kt == 0), stop=(kt == KT - 1 and not with_vb))
            if with_kb:
                nc.tensor.matmul(kps[:], ones_sb[:], kb_sb[:], start=False, stop=True)
            if with_vb:
                nc.tensor.matmul(vps[:], ones_sb[:], vb_sb[:], start=False, stop=True)

            # scatter for the previous tile (keeps PE dense: its wv is ready)
            if pending is not None:
                emit_scatter(*pending)
            if tt >= 10:
                i0 = (tt - 10) * 3
                for stt, sgt, st in st_jobs[i0:i0 + 3]:
                    nc.gpsimd.dma_start(
                        st[:], STmat[sgt * P:(sgt + 1) * P, stt * P:(stt + 1) * P])
            for kind, arg in enqueue_at.get(tt, []):
                mid_queue.append((kind, arg))
            for _ in range(2):
                if mid_queue:
                    kind, arg = mid_queue.pop(0)
                    mk = {"cvt": job_convT, "A": job_A, "out": job_out}[kind]
                    mk(arg, ps_mid, "mid")()

            # ||k|| per head
            ksq = smpool.tile([P, CLOC], F32, tag="ksq")
            nc.scalar.activation(ksq[:], kps[:], mybir.ActivationFunctionType.Square)
            km2 = smpool.tile([P, HLOC], F32, tag="km2")
            nc.vector.reduce_sum(km2[:], ksq[:].rearrange("p (h d) -> p h d", d=HD),
                                 axis=mybir.AxisListType.X)
            km = smpool.tile([P, HLOC], F32, tag="km")
            nc.scalar.sqrt(km[:], km2[:])

            # wv = v * ||k||, one DVE op via stride-0 broadcast of km
            wv = wvpool.tile([P, CLOC], F32R, tag="wv")
            nc.vector.tensor_tensor(
                wv[:].rearrange("p (h d) -> p h d", d=HD),
                vps[:].rearrange("p (h d) -> p h d", d=HD),
                km[:].unsqueeze(2).broadcast_to((P, HLOC, HD)),
                mybir.AluOpType.mult)
            pending = (tt, wv)

        emit_scatter(*pending)
        ph1.close()

        # flush any queued mid jobs (still inside ph1 pools)
        while mid_queue:
            kind, arg = mid_queue.pop(0)
            mk = {"cvt": job_convT, "A": job_A, "out": job_out}[kind]
            mk(arg, ps_mid, "mid")()
        ph1.close()

        # ---- tail: circular-wrap convT tiles 0,1 -> A -> remaining tokens ----
        ph2 = ExitStack()
        ps_t = ph2.enter_context(tc.tile_pool(name="ps_t", bufs=3, space="PSUM"))
        done = {t for jobs in enqueue_at.values() for k, t in jobs if k == "out"}
        for gp in (0, 1):
            job_convT(gp, ps_t, "tmid")()
            job_A(gp, ps_t, "tmid")()
        for t in range(TT):
            if t not in done:
                job_out(t, ps_t, "tmid")()
        ph2.close()

    nc.compile()
    return nc


_PROGRAM_CACHE = {}


def _get_program(with_kb, with_vb):
    key = (with_kb, with_vb)
    if key not in _PROGRAM_CACHE:
        _, _, _, _, sp, tg, cb = _plans()
        _PROGRAM_CACHE[key] = _build_program(with_kb, with_vb, sp, tg, cb)
    return _PROGRAM_CACHE[key]


def kernel(x, q_w, q_b, k_w, k_b, v_w, v_b, out_w, out_b):
    global LAST_RESULT
    x = np.asarray(x, dtype=np.float32)
    k_w = np.asarray(k_w, dtype=np.float32)
    k_b = np.asarray(k_b, dtype=np.float32)
    v_w = np.asarray(v_w, dtype=np.float32)
    v_b = np.asarray(v_b, dtype=np.float32)
    out_w = np.asarray(out_w, dtype=np.float32)
    out_b = np.asarray(out_b, dtype=np.float32)

    with_kb = bool(np.any(k_b))
    with_vb = bool(np.any(v_b))
    nc = _get_program(with_kb, with_vb)
    _, CTm, Smat, STm, _, _, _ = _plans()

    in_maps = []
    for c in range(NCORES):
        b, hg = c // 2, c % 2
        chs = slice(hg * CLOC, (hg + 1) * CLOC)
        m = {
            "xT": np.ascontiguousarray(x[b].T),
            "kwT": np.ascontiguousarray(k_w[chs, :].T),
            "vwT": np.ascontiguousarray(v_w[chs, :].T),
            "owT": np.ascontiguousarray(out_w[:, chs].T),
            "Smat": Smat,
            "STm": STm,
            "CTm": CTm,
        }
        if with_kb:
            m["kb"] = np.ascontiguousarray(k_b[chs][None, :])
        if with_vb:
            m["vb"] = np.ascontiguousarray(v_b[chs][None, :])
        if with_kb or with_vb:
            m["ones"] = np.ones((1, P), dtype=np.float32)
        in_maps.append(m)

    res = run_bass_kernel_spmd(nc, in_maps, core_ids=list(range(NCORES)),
                               trace=TRACE)
    LAST_RESULT = res

    out = np.empty((B, N, D), dtype=np.float32)
    for b in range(B):
        out[b] = res.results[2 * b]["out"] + res.results[2 * b + 1]["out"]
        out[b] += out_b[None, :]
    return out
